# revision 14
# baseline (speedup 1.0000x reference)
"""Dense multi-head attention (B=4, H=16, L=2048, D=64, fp32) on 8 trn2 cores.

Sharding: the 64 (batch, head) pairs split 8-per-core (core c gets batch c//2,
heads (c%2)*8 .. +8); each core computes full attention for its heads with no
cross-core communication. The host pre-transposes Q/K to d-major and appends a
ones column to V while staging per-core inputs (fp16 — S and O accumulate in
fp32 on-chip).

Per-core kernel (per head; the core is elementwise-exp bound, ~33.5M exp/core):
  - Q^T, K^T staged d-major in SBUF ([128, 2048] with the 64 d-rows duplicated
    in both partition halves so two k-tiles of the D=64-contraction QK matmul
    run concurrently via tile_position row-packing).
  - S^T 2-ktile groups [128 k, 1024 q] in fp32 PSUM (triple-buffered, 6 banks).
  - exp is split between ACT and DVE (7:1 groups — both engines run
    concurrently; the mostly-ACT split keeps the PE/ACT pipeline in lockstep):
      * "A" groups: exp(S/8) on ACT via ACTIVATE with immediate scale (exact).
      * "D" groups: DVE fast exp via the int16-Schraudolph offset-average:
          i  = rint(A*s + B)     (tensor_scalar, fp32 PSUM -> int16, per ktile)
          i2 = i + 512           (tensor_scalar int16, 4x mode)
          p  = f16(i) + f16(i2)  (tensor_tensor on bitcast-fp16 views, 2x mode)
        Calibrated A, B give |rel err| <= 1.1% (rms 0.54%, zero-mean), which
        contributes only ~3e-3 to the output gate after softmax averaging.
  - O^T_ext [65, 512] accumulates V_ext.T @ P^T in PSUM over the 16 k-tiles,
    where V_ext = [V | ones] so row 64 is the softmax denominator.
  - O^T_ext is copied to SBUF fp16 (ACT/DVE alternating) and DMA'd
    unnormalized; the host divides by the denominator row and transposes
    during unshard (removes the baseline's on-chip PE transposes + DVE
    normalize, ~100us of engine time).
  - PV matmuls are emitted PV_LAG groups behind QK so the strict-FIFO PE
    queue never head-of-line blocks on exp.
"""

import numpy as np

import concourse.bass as bass
import concourse.mybir as mybir
import concourse.tile as tile
from concourse import bass_utils

B, H, L, D = 4, 16, 2048, 64
N_CORES = 8
HEADS_PER_CORE = (B * H) // N_CORES  # 8
KT = L // 128  # 16 k-tiles per head
QT = L // 512  # 4 q-tiles per head
SCALE = 1.0 / float(np.sqrt(D))  # 1/8

F32 = mybir.dt.float32
F16 = mybir.dt.float16
I16 = mybir.dt.int16
MM_DTYPE = F16

# Schraudolph constants. "D" groups use the plain int16 variant:
#   i = rint(A*s + B) as int16; p = bitcast f16(i)
# B is calibrated so the log-domain sawtooth error (log2(1+f) - f,
# f = frac) is zero-mean: B = 1024*(15 - 0.057305). |rel| <= 3.9%,
# rms 1.77%, zero-mean -- contributes ~4.6e-3 max abs to the output
# after softmax averaging (budget 8.1e-3).
EXP_A = 1024.0 * SCALE / float(np.log(2.0))  # 184.665
EXP_B = 1024.0 * (15.0 - 0.05730496)  # 15301.32

# Engine pattern per (head, q-tile), one entry per 2-ktile PSUM group:
# "A" = ACT exact exp (1.03us), "D" = DVE plain-Schraudolph fast exp
# (single tensor_scalar over [128,1024] fp32 PSUM, 1.19us). 4:4 split
# balances ACT (128 exps + 32 O^T copies ~ 150us) against DVE
# (128 exps ~ 153us), both under the PE's ~229us.
PATTERNS = [
    ["A", "D", "A", "D", "A", "D", "A", "D"],
]
FLUSH_EVERY = 1
PV_LAG = 2  # PV of group g is emitted after QK of group g+PV_LAG


def _split_sync_waits(nc):
    """This container's walrus build rejects instructions carrying more than
    one sem wait ("Too many sync wait commands" in setupSyncWait). Splitting
    is semantics-preserving: a same-engine NoOp carrying one of the waits is
    spliced in front, and the sequencer blocks on each in order."""
    for f in nc.m.functions:
        for bb in f.blocks:
            insts = bb.instructions
            out = []
            changed = False
            for inst in insts:
                si = inst.sync_info
                if si is not None and si.on_wait and len(si.on_wait) > 1:
                    waits = list(si.on_wait)
                    for j, w in enumerate(waits[:-1]):
                        nop = mybir.InstNoOp(
                            name=f"{inst.name}_sw{j}",
                            engine=inst.engine,
                            sync_info=mybir.SyncInfo(on_wait=[w], on_update=[]),
                        )
                        out.append(nop)
                    si.on_wait = [waits[-1]]
                    changed = True
                out.append(inst)
            if changed:
                insts[:] = out


def _act_exp_imm(nc, out, in_, scale):
    """ACTIVATE Exp with immediate (non-AP) bias, skipping the const-AP
    conversion bass applies for non-Copy funcs (saves a per-call SBUF
    bias read)."""
    eng = nc.scalar
    inputs = [
        eng.lower_ap(in_),
        mybir.ImmediateValue(dtype=mybir.dt.float32, value=0.0),
        mybir.ImmediateValue(dtype=mybir.dt.float32, value=float(scale)),
        mybir.ImmediateValue(dtype=mybir.dt.float32, value=0.0),
    ]
    outputs = [eng.lower_ap(out)]
    return eng.add_instruction(
        mybir.InstActivation(
            name=nc.get_next_instruction_name(),
            func=mybir.ActivationFunctionType.Exp,
            ins=inputs,
            outs=outputs,
        )
    )


def build_nc(mm_dtype=MM_DTYPE):
    nc = bass.Bass("TRN2", target_bir_lowering=False, debug=False)

    MD = mm_dtype
    qt_d = nc.dram_tensor("qt", [HEADS_PER_CORE, D, L], MD, kind="ExternalInput")
    kt_d = nc.dram_tensor("kt", [HEADS_PER_CORE, D, L], MD, kind="ExternalInput")
    v_d = nc.dram_tensor("v", [HEADS_PER_CORE, L, D + 1], MD, kind="ExternalInput")
    o_d = nc.dram_tensor("o", [HEADS_PER_CORE, D + 1, L], F16, kind="ExternalOutput")

    with tile.TileContext(nc) as tc:
        with (
            tc.tile_pool(name="consts", bufs=1) as consts,
            tc.tile_pool(name="qk", bufs=2) as qk_pool,
            tc.tile_pool(name="vx", bufs=2) as vx_pool,
            tc.tile_pool(name="pt", bufs=6) as pt_pool,
            tc.tile_pool(name="osb", bufs=2) as osb_pool,
            tc.tile_pool(name="st", bufs=3, space="PSUM") as st_pool,
            tc.tile_pool(name="otp", bufs=2, space="PSUM") as ot_pool,
        ):
            # Dummy activation so walrus's ACT table load (~2.7us) runs
            # during the first input DMAs instead of before the first real
            # exp call.
            warm = consts.tile([1, 8], F32)
            nc.vector.memset(warm[:], 0.0)
            nc.scalar.activation(warm[:], warm[:], mybir.ActivationFunctionType.Exp)

            # Warm the PE HAM clock gate (~3.4us of activity flips it from
            # 1.2 to 2.4 GHz) with dummy matmuls during the initial input
            # DMAs, so the first real matmuls run at full rate.
            wsrc = consts.tile([128, 512], MM_DTYPE)
            nc.vector.memset(wsrc[:], 0.0)
            wps = st_pool.tile([128, 1024], F32, tag="st")
            for _ in range(5):
                nc.tensor.matmul(
                    wps[:, 0:512], lhsT=wsrc[:, 0:128], rhs=wsrc[:], start=True,
                    stop=True, skip_group_check=True,
                )

            # queue of deferred PV groups: (ot, pt, vx, g, h, q)
            pv_queue = []

            def flush_pv(limit):
                while len(pv_queue) > limit:
                    ot, pt, vx, g, h, q = pv_queue.pop(0)
                    for i in range(2):
                        kt_idx = 2 * g + i
                        nc.tensor.matmul(
                            ot[:, :],
                            lhsT=vx[:, kt_idx, :],
                            rhs=pt[:, i * 512 : (i + 1) * 512],
                            start=(kt_idx == 0),
                            stop=(kt_idx == KT - 1),
                            skip_group_check=True,
                        )
                    if kt_idx == KT - 1:
                        osb = osb_pool.tile([D + 1, 512], F16)
                        nc.scalar.copy(osb[:], ot[:])
                        nc.sync.dma_start(
                            o_d.ap()[h][:, q * 512 : (q + 1) * 512], osb[:]
                        )

            for h in range(HEADS_PER_CORE):
                qt2 = qk_pool.tile([128, L], MD, tag="qt")
                kt2 = qk_pool.tile([128, L], MD, tag="kt")
                vx = vx_pool.tile([128, KT, D + 1], MD)
                v_r = v_d.ap()[h].rearrange("(t p) d -> p t d", p=128)
                # Priority order: the first QK iterations need qt/kt cols
                # 0:512 and the first PVs need vx[0:4]; then kt's remainder
                # (consumed by g=2.. QKs) before qt's (next q-tile, much
                # later). Matters for h=0 where nothing hides the DMAs.
                s0, s1 = slice(0, 512), slice(512, L)
                for half in (slice(0, 64), slice(64, 128)):
                    nc.sync.dma_start(qt2[half, s0], qt_d.ap()[h][:, s0])
                    nc.sync.dma_start(kt2[half, s0], kt_d.ap()[h][:, s0])
                nc.sync.dma_start(vx[:, 0:4, :], v_r[:, 0:4, :])
                for half in (slice(0, 64), slice(64, 128)):
                    nc.sync.dma_start(kt2[half, s1], kt_d.ap()[h][:, s1])
                for half in (slice(0, 64), slice(64, 128)):
                    nc.sync.dma_start(qt2[half, s1], qt_d.ap()[h][:, s1])
                for c in range(1, 4):
                    nc.sync.dma_start(
                        vx[:, c * 4 : (c + 1) * 4, :], v_r[:, c * 4 : (c + 1) * 4, :]
                    )

                for q in range(QT):
                    ot = ot_pool.tile([D + 1, 512], F32)
                    pattern = PATTERNS[(h * QT + q) % len(PATTERNS)]
                    for g, eng in enumerate(pattern):
                        st = st_pool.tile([128, 1024], F32, tag="st")
                        for i in range(2):
                            kt_idx = 2 * g + i
                            half = 64 * i
                            nc.tensor.matmul(
                                st[:, i * 512 : (i + 1) * 512],
                                lhsT=kt2[
                                    half : half + 64,
                                    kt_idx * 128 : (kt_idx + 1) * 128,
                                ],
                                rhs=qt2[half : half + 64, q * 512 : (q + 1) * 512],
                                start=True,
                                stop=True,
                                tile_position=(half, 0),
                            )
                        pt = pt_pool.tile([128, 1024], F16, tag="pt")
                        if eng == "A":
                            _act_exp_imm(nc, pt[:], st[:], SCALE)
                        else:
                            nc.vector.tensor_scalar(
                                pt[:].bitcast(I16),
                                st[:],
                                EXP_A,
                                EXP_B,
                                mybir.AluOpType.mult,
                                mybir.AluOpType.add,
                            )
                        pv_queue.append((ot, pt, vx, g, h, q))
                        if g % FLUSH_EVERY == FLUSH_EVERY - 1:
                            flush_pv(PV_LAG)
            flush_pv(0)
    _split_sync_waits(nc)
    return nc


def shard_inputs(query, key, value, mm_dtype=MM_DTYPE):
    """Full [B, H, L, D] inputs -> per-core input maps (host-side layout)."""
    np_dt = mybir.dt.np(mm_dtype)
    q = np.asarray(query, dtype=np.float32).reshape(B * H, L, D).astype(np_dt)
    k = np.asarray(key, dtype=np.float32).reshape(B * H, L, D).astype(np_dt)
    v = np.asarray(value, dtype=np.float32).reshape(B * H, L, D).astype(np_dt)
    ones = np.ones((HEADS_PER_CORE, L, 1), np_dt)
    in_maps = []
    for c in range(N_CORES):
        sl = slice(c * HEADS_PER_CORE, (c + 1) * HEADS_PER_CORE)
        in_maps.append(
            {
                "qt": np.ascontiguousarray(q[sl].transpose(0, 2, 1)),
                "kt": np.ascontiguousarray(k[sl].transpose(0, 2, 1)),
                "v": np.ascontiguousarray(np.concatenate([v[sl], ones], axis=-1)),
            }
        )
    return in_maps


def unshard(results):
    """Per-core unnormalized [heads, D+1, L] outputs -> full [B, L, H*D]."""
    o = np.concatenate([r["o"] for r in results], axis=0)  # [B*H, D+1, L]
    o = o[:, :D, :].astype(np.float32) / o[:, D : D + 1, :].astype(np.float32)
    # [B*H, D, L] -> [B, L, H*D]
    o = o.reshape(B, H, D, L).transpose(0, 3, 1, 2).reshape(B, L, H * D)
    return np.ascontiguousarray(o.astype(np.float32))


_NC_CACHE = {}


def run(query, key, value, trace=False, mm_dtype=MM_DTYPE):
    key_ = mm_dtype
    if key_ not in _NC_CACHE:
        _NC_CACHE[key_] = build_nc(mm_dtype)
    nc = _NC_CACHE[key_]
    in_maps = shard_inputs(query, key, value, mm_dtype)
    res = bass_utils.run_bass_kernel_spmd(
        nc, in_maps, core_ids=list(range(N_CORES)), trace=trace
    )
    return unshard(res.results), res


def kernel(query, key, value, mask=None, to_q=None, to_k=None):
    out, _ = run(query, key, value, trace=False)
    return out


if __name__ == "__main__":
    rng = np.random.default_rng(0)
    q = rng.normal(size=(B, H, L, D)).astype(np.float32)
    k = rng.normal(size=(B, H, L, D)).astype(np.float32)
    v = rng.normal(size=(B, H, L, D)).astype(np.float32)
    out = kernel(q, k, v)
    print("out", out.shape, out.dtype)



# revision 15
# speedup vs baseline: 1.0050x; 1.0050x over previous
"""Dense multi-head attention (B=4, H=16, L=2048, D=64, fp32) on 8 trn2 cores.

Sharding: the 64 (batch, head) pairs split 8-per-core (core c gets batch c//2,
heads (c%2)*8 .. +8); each core computes full attention for its heads with no
cross-core communication. The host pre-transposes Q/K to d-major and appends a
ones column to V while staging per-core inputs (fp16 — S and O accumulate in
fp32 on-chip).

Per-core kernel (per head; the core is elementwise-exp bound, ~33.5M exp/core):
  - Q^T, K^T staged d-major in SBUF ([128, 2048] with the 64 d-rows duplicated
    in both partition halves so two k-tiles of the D=64-contraction QK matmul
    run concurrently via tile_position row-packing).
  - S^T 2-ktile groups [128 k, 1024 q] in fp32 PSUM (triple-buffered, 6 banks).
  - exp is split between ACT and DVE (7:1 groups — both engines run
    concurrently; the mostly-ACT split keeps the PE/ACT pipeline in lockstep):
      * "A" groups: exp(S/8) on ACT via ACTIVATE with immediate scale (exact).
      * "D" groups: DVE fast exp via the int16-Schraudolph offset-average:
          i  = rint(A*s + B)     (tensor_scalar, fp32 PSUM -> int16, per ktile)
          i2 = i + 512           (tensor_scalar int16, 4x mode)
          p  = f16(i) + f16(i2)  (tensor_tensor on bitcast-fp16 views, 2x mode)
        Calibrated A, B give |rel err| <= 1.1% (rms 0.54%, zero-mean), which
        contributes only ~3e-3 to the output gate after softmax averaging.
  - O^T_ext [65, 512] accumulates V_ext.T @ P^T in PSUM over the 16 k-tiles,
    where V_ext = [V | ones] so row 64 is the softmax denominator.
  - O^T_ext is copied to SBUF fp16 (ACT/DVE alternating) and DMA'd
    unnormalized; the host divides by the denominator row and transposes
    during unshard (removes the baseline's on-chip PE transposes + DVE
    normalize, ~100us of engine time).
  - PV matmuls are emitted PV_LAG groups behind QK so the strict-FIFO PE
    queue never head-of-line blocks on exp.
"""

import numpy as np

import concourse.bass as bass
import concourse.mybir as mybir
import concourse.tile as tile
from concourse import bass_utils

B, H, L, D = 4, 16, 2048, 64
N_CORES = 8
HEADS_PER_CORE = (B * H) // N_CORES  # 8
KT = L // 128  # 16 k-tiles per head
QT = L // 512  # 4 q-tiles per head
SCALE = 1.0 / float(np.sqrt(D))  # 1/8

F32 = mybir.dt.float32
F16 = mybir.dt.float16
I16 = mybir.dt.int16
MM_DTYPE = F16

# Schraudolph constants. "D" groups use the plain int16 variant:
#   i = rint(A*s + B) as int16; p = bitcast f16(i)
# B is calibrated so the log-domain sawtooth error (log2(1+f) - f,
# f = frac) is zero-mean: B = 1024*(15 - 0.057305). |rel| <= 3.9%,
# rms 1.77%, zero-mean -- contributes ~4.6e-3 max abs to the output
# after softmax averaging (budget 8.1e-3).
EXP_A = 1024.0 * SCALE / float(np.log(2.0))  # 184.665
EXP_B = 1024.0 * (15.0 - 0.05730496)  # 15301.32

# Engine pattern per (head, q-tile), one entry per 2-ktile PSUM group:
# "A" = ACT exact exp (1.03us), "D" = DVE plain-Schraudolph fast exp
# (single tensor_scalar over [128,1024] fp32 PSUM, 1.19us). 4:4 split
# balances ACT (128 exps + 32 O^T copies ~ 150us) against DVE
# (128 exps ~ 153us), both under the PE's ~229us.
PATTERNS = [
    ["A", "D", "A", "D", "A", "D", "A", "D"],
]
FLUSH_EVERY = 1
PV_LAG = 3  # PV of group g is emitted after QK of group g+PV_LAG


def _split_sync_waits(nc):
    """This container's walrus build rejects instructions carrying more than
    one sem wait ("Too many sync wait commands" in setupSyncWait). Splitting
    is semantics-preserving: a same-engine NoOp carrying one of the waits is
    spliced in front, and the sequencer blocks on each in order."""
    for f in nc.m.functions:
        for bb in f.blocks:
            insts = bb.instructions
            out = []
            changed = False
            for inst in insts:
                si = inst.sync_info
                if si is not None and si.on_wait and len(si.on_wait) > 1:
                    waits = list(si.on_wait)
                    for j, w in enumerate(waits[:-1]):
                        nop = mybir.InstNoOp(
                            name=f"{inst.name}_sw{j}",
                            engine=inst.engine,
                            sync_info=mybir.SyncInfo(on_wait=[w], on_update=[]),
                        )
                        out.append(nop)
                    si.on_wait = [waits[-1]]
                    changed = True
                out.append(inst)
            if changed:
                insts[:] = out


def _act_exp_imm(nc, out, in_, scale):
    """ACTIVATE Exp with immediate (non-AP) bias, skipping the const-AP
    conversion bass applies for non-Copy funcs (saves a per-call SBUF
    bias read)."""
    eng = nc.scalar
    inputs = [
        eng.lower_ap(in_),
        mybir.ImmediateValue(dtype=mybir.dt.float32, value=0.0),
        mybir.ImmediateValue(dtype=mybir.dt.float32, value=float(scale)),
        mybir.ImmediateValue(dtype=mybir.dt.float32, value=0.0),
    ]
    outputs = [eng.lower_ap(out)]
    return eng.add_instruction(
        mybir.InstActivation(
            name=nc.get_next_instruction_name(),
            func=mybir.ActivationFunctionType.Exp,
            ins=inputs,
            outs=outputs,
        )
    )


def build_nc(mm_dtype=MM_DTYPE):
    nc = bass.Bass("TRN2", target_bir_lowering=False, debug=False)

    MD = mm_dtype
    qt_d = nc.dram_tensor("qt", [HEADS_PER_CORE, D, L], MD, kind="ExternalInput")
    kt_d = nc.dram_tensor("kt", [HEADS_PER_CORE, D, L], MD, kind="ExternalInput")
    v_d = nc.dram_tensor("v", [HEADS_PER_CORE, L, D + 1], MD, kind="ExternalInput")
    o_d = nc.dram_tensor("o", [HEADS_PER_CORE, D + 1, L], F16, kind="ExternalOutput")

    with tile.TileContext(nc) as tc:
        with (
            tc.tile_pool(name="consts", bufs=1) as consts,
            tc.tile_pool(name="qk", bufs=2) as qk_pool,
            tc.tile_pool(name="vx", bufs=2) as vx_pool,
            tc.tile_pool(name="pt", bufs=6) as pt_pool,
            tc.tile_pool(name="osb", bufs=2) as osb_pool,
            tc.tile_pool(name="st", bufs=3, space="PSUM") as st_pool,
            tc.tile_pool(name="otp", bufs=2, space="PSUM") as ot_pool,
        ):
            # Dummy activation so walrus's ACT table load (~2.7us) runs
            # during the first input DMAs instead of before the first real
            # exp call.
            warm = consts.tile([1, 8], F32)
            nc.vector.memset(warm[:], 0.0)
            nc.scalar.activation(warm[:], warm[:], mybir.ActivationFunctionType.Exp)

            # Warm the PE HAM clock gate (~3.4us of activity flips it from
            # 1.2 to 2.4 GHz) with dummy matmuls during the initial input
            # DMAs, so the first real matmuls run at full rate.
            wsrc = consts.tile([128, 512], MM_DTYPE)
            nc.vector.memset(wsrc[:], 0.0)
            wps = st_pool.tile([128, 1024], F32, tag="st")
            for _ in range(5):
                nc.tensor.matmul(
                    wps[:, 0:512], lhsT=wsrc[:, 0:128], rhs=wsrc[:], start=True,
                    stop=True, skip_group_check=True,
                )

            # queue of deferred PV groups: (ot, pt, vx, g, h, q)
            pv_queue = []

            def flush_pv(limit):
                while len(pv_queue) > limit:
                    ot, pt, vx, g, h, q = pv_queue.pop(0)
                    for i in range(2):
                        kt_idx = 2 * g + i
                        nc.tensor.matmul(
                            ot[:, :],
                            lhsT=vx[:, kt_idx, :],
                            rhs=pt[:, i * 512 : (i + 1) * 512],
                            start=(kt_idx == 0),
                            stop=(kt_idx == KT - 1),
                            skip_group_check=True,
                        )
                    if kt_idx == KT - 1:
                        osb = osb_pool.tile([D + 1, 512], F16)
                        nc.scalar.copy(osb[:], ot[:])
                        nc.sync.dma_start(
                            o_d.ap()[h][:, q * 512 : (q + 1) * 512], osb[:]
                        )

            for h in range(HEADS_PER_CORE):
                qt2 = qk_pool.tile([128, L], MD, tag="qt")
                kt2 = qk_pool.tile([128, L], MD, tag="kt")
                vx = vx_pool.tile([128, KT, D + 1], MD)
                v_r = v_d.ap()[h].rearrange("(t p) d -> p t d", p=128)
                # Priority order: the first QK iterations need qt/kt cols
                # 0:512 and the first PVs need vx[0:4]; then kt's remainder
                # (consumed by g=2.. QKs) before qt's (next q-tile, much
                # later). Matters for h=0 where nothing hides the DMAs.
                s0, s1 = slice(0, 512), slice(512, L)
                for half in (slice(0, 64), slice(64, 128)):
                    nc.sync.dma_start(qt2[half, s0], qt_d.ap()[h][:, s0])
                    nc.sync.dma_start(kt2[half, s0], kt_d.ap()[h][:, s0])
                nc.sync.dma_start(vx[:, 0:4, :], v_r[:, 0:4, :])
                for half in (slice(0, 64), slice(64, 128)):
                    nc.sync.dma_start(kt2[half, s1], kt_d.ap()[h][:, s1])
                for half in (slice(0, 64), slice(64, 128)):
                    nc.sync.dma_start(qt2[half, s1], qt_d.ap()[h][:, s1])
                for c in range(1, 4):
                    nc.sync.dma_start(
                        vx[:, c * 4 : (c + 1) * 4, :], v_r[:, c * 4 : (c + 1) * 4, :]
                    )

                for q in range(QT):
                    ot = ot_pool.tile([D + 1, 512], F32)
                    pattern = PATTERNS[(h * QT + q) % len(PATTERNS)]
                    for g, eng in enumerate(pattern):
                        st = st_pool.tile([128, 1024], F32, tag="st")
                        for i in range(2):
                            kt_idx = 2 * g + i
                            half = 64 * i
                            nc.tensor.matmul(
                                st[:, i * 512 : (i + 1) * 512],
                                lhsT=kt2[
                                    half : half + 64,
                                    kt_idx * 128 : (kt_idx + 1) * 128,
                                ],
                                rhs=qt2[half : half + 64, q * 512 : (q + 1) * 512],
                                start=True,
                                stop=True,
                                tile_position=(half, 0),
                            )
                        pt = pt_pool.tile([128, 1024], F16, tag="pt")
                        if eng == "A":
                            _act_exp_imm(nc, pt[:], st[:], SCALE)
                        else:
                            nc.vector.tensor_scalar(
                                pt[:].bitcast(I16),
                                st[:],
                                EXP_A,
                                EXP_B,
                                mybir.AluOpType.mult,
                                mybir.AluOpType.add,
                            )
                        pv_queue.append((ot, pt, vx, g, h, q))
                        if g % FLUSH_EVERY == FLUSH_EVERY - 1:
                            flush_pv(PV_LAG)
            flush_pv(0)
    _split_sync_waits(nc)
    return nc


def shard_inputs(query, key, value, mm_dtype=MM_DTYPE):
    """Full [B, H, L, D] inputs -> per-core input maps (host-side layout)."""
    np_dt = mybir.dt.np(mm_dtype)
    q = np.asarray(query, dtype=np.float32).reshape(B * H, L, D).astype(np_dt)
    k = np.asarray(key, dtype=np.float32).reshape(B * H, L, D).astype(np_dt)
    v = np.asarray(value, dtype=np.float32).reshape(B * H, L, D).astype(np_dt)
    ones = np.ones((HEADS_PER_CORE, L, 1), np_dt)
    in_maps = []
    for c in range(N_CORES):
        sl = slice(c * HEADS_PER_CORE, (c + 1) * HEADS_PER_CORE)
        in_maps.append(
            {
                "qt": np.ascontiguousarray(q[sl].transpose(0, 2, 1)),
                "kt": np.ascontiguousarray(k[sl].transpose(0, 2, 1)),
                "v": np.ascontiguousarray(np.concatenate([v[sl], ones], axis=-1)),
            }
        )
    return in_maps


def unshard(results):
    """Per-core unnormalized [heads, D+1, L] outputs -> full [B, L, H*D]."""
    o = np.concatenate([r["o"] for r in results], axis=0)  # [B*H, D+1, L]
    o = o[:, :D, :].astype(np.float32) / o[:, D : D + 1, :].astype(np.float32)
    # [B*H, D, L] -> [B, L, H*D]
    o = o.reshape(B, H, D, L).transpose(0, 3, 1, 2).reshape(B, L, H * D)
    return np.ascontiguousarray(o.astype(np.float32))


_NC_CACHE = {}


def run(query, key, value, trace=False, mm_dtype=MM_DTYPE):
    key_ = mm_dtype
    if key_ not in _NC_CACHE:
        _NC_CACHE[key_] = build_nc(mm_dtype)
    nc = _NC_CACHE[key_]
    in_maps = shard_inputs(query, key, value, mm_dtype)
    res = bass_utils.run_bass_kernel_spmd(
        nc, in_maps, core_ids=list(range(N_CORES)), trace=trace
    )
    return unshard(res.results), res


def kernel(query, key, value, mask=None, to_q=None, to_k=None):
    out, _ = run(query, key, value, trace=False)
    return out


if __name__ == "__main__":
    rng = np.random.default_rng(0)
    q = rng.normal(size=(B, H, L, D)).astype(np.float32)
    k = rng.normal(size=(B, H, L, D)).astype(np.float32)
    v = rng.normal(size=(B, H, L, D)).astype(np.float32)
    out = kernel(q, k, v)
    print("out", out.shape, out.dtype)

